# revision 1
# baseline (speedup 1.0000x reference)
"""Trainium2 Bass kernel for nn_EpisodeMultiheadAttentionBlock.

Data-parallel over batch: each of the 8 NeuronCores handles one batch element
(B=8). Per core, a fused attention block:

  q/k/v projections (f32r matmuls) -> causal+pad+eye masked attention with a
  max-free softmax (scores bounded), computed in BOTH [q,k] and [k,q]
  orientations to avoid on-device transposes -> context -> out projection ->
  LayerNorm -> residual.  attn_weights (head-mean of probs) accumulate on the
  PE via diag(1/(H*l)) matmuls.

Masking is additive (-2^96) built from:
  - rank-1 K=1 matmuls for the key-padding mask / fully-masked blocks
  - host-precomputed [128,128] diagonal-block masks (causal+pad+eye) added on
    the vector engine directly into PSUM.
Causal structure skips fully-masked score blocks entirely.
"""
import sys

if "/opt/trn_rl_repo" not in sys.path:
    sys.path.insert(0, "/opt/trn_rl_repo")

import numpy as np
import ml_dtypes

import concourse.bass as bass
import concourse.tile as tile
from concourse import bacc, mybir
from concourse.bass_utils import run_bass_kernel_spmd

F32 = mybir.dt.float32
F32R = mybir.dt.float32r
BF16 = mybir.dt.bfloat16
Act = mybir.ActivationFunctionType

B = 8
L = 1024
E = 1024
H = 16
D = E // H          # 64
P = 128
NT = L // P         # 8
NE = E // P         # 8
HP = H // 2         # head pairs
BIG = float(2 ** 96)
LN_EPS = 1e-5
SCALE = 1.0 / np.sqrt(D)  # 0.125


def _chunks(start, end, step=512):
    out = []
    while start < end:
        out.append((start, min(start + step, end)))
        start += step
    return out


def _chunks_aligned(start, end, step=512):
    """Chunks breaking at multiples of `step` (psum bank grid)."""
    out = []
    while start < end:
        nxt = min((start // step + 1) * step, end)
        out.append((start, nxt))
        start = nxt
    return out


PSSM, PSA, PST, PSC, PEXP, PTP_B = 2, 1, 2, 1, 4, 4


def build():
    nc = bacc.Bacc("TRN2", target_bir_lowering=False, debug=False, num_devices=B)

    xt_d = nc.dram_tensor("xt", [E, L], F32R, kind="ExternalInput").ap()
    xres_d = nc.dram_tensor("xres", [L, E], F32, kind="ExternalInput").ap()
    wq_d = nc.dram_tensor("wq", [E, E], F32R, kind="ExternalInput").ap()
    wk_d = nc.dram_tensor("wk", [E, E], F32R, kind="ExternalInput").ap()
    wv_d = nc.dram_tensor("wv", [E, E], F32R, kind="ExternalInput").ap()
    wo_d = nc.dram_tensor("wo", [E, E], F32R, kind="ExternalInput").ap()
    bq_d = nc.dram_tensor("bq", [E], BF16, kind="ExternalInput").ap()
    bk_d = nc.dram_tensor("bk", [E], BF16, kind="ExternalInput").ap()
    bv_d = nc.dram_tensor("bv", [E], BF16, kind="ExternalInput").ap()
    bo_d = nc.dram_tensor("bo", [E], BF16, kind="ExternalInput").ap()
    pad_d = nc.dram_tensor("pad", [L], BF16, kind="ExternalInput").ap()
    madd_d = nc.dram_tensor("madd", [NT, P, 512], BF16, kind="ExternalInput").ap()
    maddt_d = nc.dram_tensor("maddt", [NT, P, 512], BF16, kind="ExternalInput").ap()
    g_d = nc.dram_tensor("g", [E], F32, kind="ExternalInput").ap()

    out_d = nc.dram_tensor("out", [L, E], F32, kind="ExternalOutput").ap()
    attn_d = nc.dram_tensor("attn", [L, L], F32, kind="ExternalOutput").ap()

    with tile.TileContext(nc) as tc:
        # ---------- long-lived constants ----------
        with (
            tc.tile_pool(name="consts", bufs=1) as consts,
            tc.tile_pool(name="dscratch", bufs=1, space="DRAM") as dscratch,
        ):
            sdram = dscratch.tile([H, L], F32)
            ctxf_sb = consts.tile([P, NE, L], BF16)   # resident unscaled ctx^T
            ones_bf = consts.tile([1, L], BF16)
            nc.vector.memset(ones_bf[:], 1.0)
            negbig = consts.tile([1, P], BF16)
            nc.vector.memset(negbig[:], -BIG)
            pad_sb = consts.tile([1, L], BF16)
            nc.sync.dma_start(out=pad_sb[:], in_=pad_d.rearrange("(o n) -> o n", o=1))
            bq_sb = consts.tile([1, E], BF16)
            nc.sync.dma_start(out=bq_sb[:], in_=bq_d.rearrange("(o n) -> o n", o=1))
            bk_sb = consts.tile([1, E], BF16)
            nc.sync.dma_start(out=bk_sb[:], in_=bk_d.rearrange("(o n) -> o n", o=1))
            bv_sb = consts.tile([1, E], BF16)
            nc.sync.dma_start(out=bv_sb[:], in_=bv_d.rearrange("(o n) -> o n", o=1))
            bo_sb = consts.tile([1, E], BF16)
            nc.sync.dma_start(out=bo_sb[:], in_=bo_d.rearrange("(o n) -> o n", o=1))
            madd_sb = consts.tile([P, NT, 512], BF16)
            nc.sync.dma_start(out=madd_sb[:], in_=madd_d.rearrange("t p j -> p t j"))
            maddt_sb = consts.tile([P, NT, 512], BF16)
            nc.sync.dma_start(out=maddt_sb[:], in_=maddt_d.rearrange("t p j -> p t j"))
            g_bcast = consts.tile([P, E], F32)
            nc.sync.dma_start(
                out=g_bcast[:],
                in_=bass.AP(tensor=g_d.tensor, offset=0, ap=[[0, P], [1, E]]),
            )
            eps_sb = consts.tile([P, 1], F32)
            nc.vector.memset(eps_sb[:], LN_EPS)
            idn = consts.tile([P, P], BF16)
            nc.vector.memset(idn[:], 1.0)
            idnm = consts.tile([P, P], BF16)
            nc.gpsimd.affine_select(
                out=idnm[:], in_=idn[:],
                pattern=[[-1, P]], base=0, channel_multiplier=1,
                compare_op=mybir.AluOpType.is_equal, fill=0.0,
            )

            # ---------- persistent activations ----------
            with tc.tile_pool(name="acts", bufs=1) as acts:
                qt_sb = acts.tile([P, NE, L], F32R)   # [e' in tile, me, t]
                kt_sb = acts.tile([P, NE, L], F32R)
                v_sb = acts.tile([P, NT, E], BF16)    # [t in tile, mt, e']

                # ================= phase 1: projections =================
                with (
                    tc.tile_pool(name="p1", bufs=1) as p1,
                    tc.tile_pool(name="wstr", bufs=6) as wstr,
                    tc.tile_pool(name="ps1", bufs=4, space="PSUM") as ps1,
                ):
                    xt_sb = p1.tile([P, NE, L], F32R)
                    nc.sync.dma_start(
                        out=xt_sb[:], in_=xt_d.rearrange("(ke p) t -> p ke t", p=P)
                    )
                    wv_sb = p1.tile([P, NE, E], F32R)
                    nc.sync.dma_start(
                        out=wv_sb[:], in_=wv_d.rearrange("(ke p) e -> p ke e", p=P)
                    )

                    for w_d, b_sb, dst in ((wq_d, bq_sb, qt_sb), (wk_d, bk_sb, kt_sb)):
                        for me in range(NE):
                            psc = [
                                ps1.tile([P, 512], F32, name=f"psqk{me}c{c}", tag=f"psqk{c}")
                                for c in range(2)
                            ]
                            for ke in range(NE):
                                wt = wstr.tile([P, P], F32R, name=f"wt{me}k{ke}", tag="wt")
                                nc.sync.dma_start(
                                    out=wt[:],
                                    in_=w_d[ke * P:(ke + 1) * P, me * P:(me + 1) * P],
                                )
                                for c in range(2):
                                    nc.tensor.matmul(
                                        psc[c][:], wt[:], xt_sb[:, ke, c * 512:(c + 1) * 512],
                                        start=(ke == 0), stop=False,
                                    )
                            for c in range(2):
                                nc.tensor.matmul(
                                    psc[c][:],
                                    b_sb[0:1, me * P:(me + 1) * P],
                                    ones_bf[0:1, 0:512],
                                    start=False, stop=True,
                                )
                                nc.scalar.copy(
                                    out=dst[:, me, c * 512:(c + 1) * 512], in_=psc[c][:]
                                )

                    for mt in range(NT):
                        psc = [
                            ps1.tile([P, 512], F32, name=f"psv{mt}c{c}", tag=f"psqk{c}")
                            for c in range(2)
                        ]
                        for ke in range(NE):
                            for c in range(2):
                                nc.tensor.matmul(
                                    psc[c][:],
                                    xt_sb[:, ke, mt * P:(mt + 1) * P],
                                    wv_sb[:, ke, c * 512:(c + 1) * 512],
                                    start=(ke == 0), stop=False,
                                )
                        for c in range(2):
                            nc.tensor.matmul(
                                psc[c][:],
                                ones_bf[0:1, 0:P],
                                bv_sb[0:1, c * 512:(c + 1) * 512],
                                start=False, stop=True,
                            )
                            nc.scalar.copy(
                                out=v_sb[:, mt, c * 512:(c + 1) * 512], in_=psc[c][:]
                            )

                # ======== phases 2a+2b interleaved: A-path + ST/ctx ========
                with (
                    tc.tile_pool(name="p2a", bufs=1) as p2a,
                    tc.tile_pool(name="pexp", bufs=PEXP) as pexp,
                    tc.tile_pool(name="small", bufs=8) as small,
                    tc.tile_pool(name="aout", bufs=2) as aoutp,
                    tc.tile_pool(name="ptp", bufs=PTP_B) as ptp,
                    tc.tile_pool(name="ctxu", bufs=3) as ctxup,
                    tc.tile_pool(name="psSm", bufs=PSSM, space="PSUM") as psSm,
                    tc.tile_pool(name="psA", bufs=PSA, space="PSUM") as psA,
                    tc.tile_pool(name="psT", bufs=PST, space="PSUM") as psT,
                    tc.tile_pool(name="psC", bufs=PSC, space="PSUM") as psC,
                ):
                    s_all = p2a.tile([P, H, NT], F32)
                    zeros = p2a.tile([P, L - P], F32)
                    nc.vector.memset(zeros[:], 0.0)
                    for qt in range(NT - 1):
                        nc.sync.dma_start(
                            out=attn_d[qt * P:(qt + 1) * P, (qt + 1) * P:L],
                            in_=zeros[:, 0:L - (qt + 1) * P],
                        )

                    def block_2a(qt):
                        W = (qt + 1) * P
                        dc = qt * P
                        a_ps = psA.tile([P, L], F32, name=f"aps{qt}", tag="aps")
                        for h in range(H):
                            po = (h % 2) * 64
                            qslice = qt_sb[po:po + 64, h // 2, qt * P:(qt + 1) * P]
                            p_t = pexp.tile([P, L], BF16, name=f"pt{qt}h{h}", tag="pt")
                            ch = _chunks(0, W)
                            l_parts = small.tile(
                                [P, len(ch)], F32, name=f"lp{qt}h{h}", tag="lp"
                            )
                            for ci, (cs, ce) in enumerate(ch):
                                s_ps = psSm.tile(
                                    [P, 512], F32, name=f"sps{qt}h{h}c{ci}", tag="sps"
                                )
                                w = ce - cs
                                if ce <= dc:
                                    nc.tensor.matmul(
                                        s_ps[:, 0:w], qslice,
                                        kt_sb[po:po + 64, h // 2, cs:ce],
                                        start=True, stop=False,
                                    )
                                    nc.tensor.matmul(
                                        s_ps[:, 0:w],
                                        ones_bf[0:1, 0:P],
                                        pad_sb[0:1, cs:ce],
                                        start=False, stop=True,
                                    )
                                else:
                                    nc.tensor.matmul(
                                        s_ps[:, 0:w], qslice,
                                        kt_sb[po:po + 64, h // 2, cs:ce],
                                        start=True, stop=True,
                                    )
                                    nc.vector.tensor_add(
                                        out=s_ps[:, 0:w], in0=s_ps[:, 0:w],
                                        in1=madd_sb[:, qt, 0:w],
                                    )
                                nc.scalar.activation(
                                    out=p_t[:, cs:ce], in_=s_ps[:, 0:w],
                                    func=Act.Exp, scale=SCALE,
                                    accum_out=l_parts[:, ci:ci + 1],
                                )
                            l16 = small.tile([P, 1], F32, name=f"l16{qt}h{h}", tag="l16")
                            if len(ch) > 1:
                                l_s = small.tile([P, 1], F32, name=f"ls{qt}h{h}", tag="ls")
                                nc.vector.tensor_reduce(
                                    out=l_s[:], in_=l_parts[:],
                                    axis=mybir.AxisListType.X, op=mybir.AluOpType.add,
                                )
                                nc.vector.tensor_scalar_mul(l16[:], l_s[:], 16.0)
                            else:
                                nc.vector.tensor_scalar_mul(l16[:], l_parts[:, 0:1], 16.0)
                            s_col = small.tile([P, 1], F32, name=f"sc{qt}h{h}", tag="sc")
                            nc.vector.reciprocal(out=s_col[:], in_=l16[:])
                            nc.vector.tensor_copy(
                                out=s_all[:, h, qt:qt + 1], in_=s_col[:]
                            )
                            dg = small.tile([P, P], BF16, name=f"dg{qt}h{h}", tag="dg")
                            nc.vector.tensor_scalar_mul(dg[:], idnm[:], s_col[:])
                            for (cs, ce) in _chunks(0, W):
                                nc.tensor.matmul(
                                    a_ps[:, cs:ce], dg[:], p_t[:, cs:ce],
                                    start=(h == 0), stop=(h == H - 1),
                                )
                        a_out = aoutp.tile([P, L], F32, name=f"aout{qt}", tag="aout")
                        nc.scalar.copy(out=a_out[:, 0:W], in_=a_ps[:, 0:W])
                        nc.sync.dma_start(
                            out=attn_d[qt * P:(qt + 1) * P, 0:W], in_=a_out[:, 0:W]
                        )

                    def block_2b(h):
                        ctx_ps = psC.tile([64, L], F32, name=f"ctxps{h}", tag="ctxps")
                        po = (h % 2) * 64
                        for kt in range(NT):
                            c0 = 512 * (kt // 4)
                            d0 = kt * P
                            kslice = kt_sb[po:po + 64, h // 2, kt * P:(kt + 1) * P]
                            pt_t = ptp.tile(
                                [P, L], BF16, name=f"ptt{h}k{kt}", tag="ptt"
                            )
                            if c0 < d0:
                                nc.vector.memset(pt_t[:, c0:d0], 0.0)
                            for ci, (cs, ce) in enumerate(_chunks(d0, L)):
                                st_ps = psT.tile(
                                    [P, 512], F32, name=f"stp{h}k{kt}c{ci}", tag="stp"
                                )
                                w = ce - cs
                                if ci == 0:
                                    nc.tensor.matmul(
                                        st_ps[:, 0:w], kslice,
                                        qt_sb[po:po + 64, h // 2, cs:ce],
                                        start=True, stop=True,
                                    )
                                    nc.vector.tensor_add(
                                        out=st_ps[:, 0:w], in0=st_ps[:, 0:w],
                                        in1=maddt_sb[:, kt, 0:w],
                                    )
                                else:
                                    nc.tensor.matmul(
                                        st_ps[:, 0:w], kslice,
                                        qt_sb[po:po + 64, h // 2, cs:ce],
                                        start=True, stop=False,
                                    )
                                    nc.tensor.matmul(
                                        st_ps[:, 0:w],
                                        pad_sb[0:1, kt * P:(kt + 1) * P],
                                        ones_bf[0:1, cs:ce],
                                        start=False, stop=True,
                                    )
                                nc.scalar.activation(
                                    out=pt_t[:, cs:ce], in_=st_ps[:, 0:w],
                                    func=Act.Exp, scale=SCALE,
                                )
                            for (cs, ce) in _chunks(0, L):
                                if ce <= c0:
                                    continue
                                n_kt = min(NT, (ce + P - 1) // P)
                                nc.tensor.matmul(
                                    ctx_ps[:, cs:ce],
                                    v_sb[:, kt, h * D:(h + 1) * D],
                                    pt_t[:, cs:ce],
                                    start=(kt == 0),
                                    stop=(kt == n_kt - 1),
                                )
                        ctxu = ctxup.tile([64, L], BF16, name=f"ctxu{h}", tag="ctxu")
                        po2 = (h % 2) * 64
                        for (cs, ce) in _chunks(0, L):
                            nc.scalar.copy(out=ctxu[:, cs:ce], in_=ctx_ps[:, cs:ce])
                            nc.sync.dma_start(
                                out=ctxf_sb[po2:po2 + 64, h // 2, cs:ce],
                                in_=ctxu[:, cs:ce],
                            )

                    for i in range(NT):
                        block_2a(i)
                        block_2b(2 * i)
                        block_2b(2 * i + 1)
                    nc.sync.dma_start(
                        out=bass.AP(
                            tensor=sdram.tensor, offset=sdram.offset,
                            ap=[[1, P], [L, H], [P, NT]],
                        ),
                        in_=s_all[:],
                    )

            # ================= phase 3: out-proj + LN + residual =================
            with (
                tc.tile_pool(name="p3", bufs=1) as p3,
                tc.tile_pool(name="xrp", bufs=2) as xrp,
                tc.tile_pool(name="znp", bufs=2) as znp,
                tc.tile_pool(name="lns", bufs=8) as lns,
                tc.tile_pool(name="ps3", bufs=4, space="PSUM") as ps3,
            ):
                ctxt_sb = p3.tile([P, NE, L], F32R)
                for ke in range(NE):
                    sbc2 = xrp.tile([P, L], F32, name=f"sbc2k{ke}", tag="sbc2")
                    nc.sync.dma_start(
                        out=sbc2[:],
                        in_=bass.AP(
                            tensor=sdram.tensor, offset=sdram.offset + 2 * ke * L,
                            ap=[[L, 2], [0, 64], [1, L]],
                        ),
                    )
                    nc.vector.scalar_tensor_tensor(
                        out=ctxt_sb[:, ke, :], in0=ctxf_sb[:, ke, :],
                        scalar=16.0, in1=sbc2[:],
                        op0=mybir.AluOpType.mult, op1=mybir.AluOpType.mult,
                    )
                wo_sb = p3.tile([P, NE, E], F32R)
                nc.sync.dma_start(
                    out=wo_sb[:], in_=wo_d.rearrange("(ke p) e -> p ke e", p=P)
                )
                for qt in range(NT):
                    psc = [
                        ps3.tile([P, 512], F32, name=f"pso{qt}c{c}", tag=f"pso{c}")
                        for c in range(2)
                    ]
                    for ke in range(NE):
                        for c in range(2):
                            nc.tensor.matmul(
                                psc[c][:],
                                ctxt_sb[:, ke, qt * P:(qt + 1) * P],
                                wo_sb[:, ke, c * 512:(c + 1) * 512],
                                start=(ke == 0), stop=False,
                            )
                    for c in range(2):
                        nc.tensor.matmul(
                            psc[c][:],
                            ones_bf[0:1, 0:P],
                            bo_sb[0:1, c * 512:(c + 1) * 512],
                            start=False, stop=True,
                        )
                    stats = lns.tile([P, 2, 6], F32, name=f"st{qt}", tag="st")
                    for c in range(2):
                        nc.vector.bn_stats(out=stats[:, c, :], in_=psc[c][:])
                    mv = lns.tile([P, 2], F32, name=f"mv{qt}", tag="mv")
                    nc.vector.bn_aggr(out=mv[:], in_=stats[:])
                    sd = lns.tile([P, 1], F32, name=f"sd{qt}", tag="sd")
                    nc.scalar.activation(
                        out=sd[:], in_=mv[:, 1:2], func=Act.Sqrt, bias=eps_sb[:],
                    )
                    rstd = lns.tile([P, 1], F32, name=f"rs{qt}", tag="rs")
                    nc.vector.reciprocal(out=rstd[:], in_=sd[:])
                    nmu = lns.tile([P, 1], F32, name=f"nm{qt}", tag="nm")
                    nc.vector.scalar_tensor_tensor(
                        out=nmu[:], in0=mv[:, 0:1], scalar=-1.0, in1=rstd[:],
                        op0=mybir.AluOpType.mult, op1=mybir.AluOpType.mult,
                    )
                    zn = znp.tile([P, E], F32, name=f"zn{qt}", tag="zn")
                    for c in range(2):
                        nc.scalar.activation(
                            out=zn[:, c * 512:(c + 1) * 512], in_=psc[c][:],
                            func=Act.Identity, bias=nmu[:], scale=rstd[:],
                        )
                    xr = xrp.tile([P, E], F32, name=f"xr{qt}", tag="xr")
                    nc.sync.dma_start(out=xr[:], in_=xres_d[qt * P:(qt + 1) * P, :])
                    nc.vector.tensor_mul(zn[:], zn[:], g_bcast[:])
                    nc.vector.tensor_add(zn[:], zn[:], xr[:])
                    nc.sync.dma_start(out=out_d[qt * P:(qt + 1) * P, :], in_=zn[:])

    nc.compile()
    return nc


_NC = None


def _get_nc():
    global _NC
    if _NC is None:
        _NC = build()
    return _NC


def _host_prep(key, key_padding_mask, in_proj_w, in_proj_b, out_w, out_b, ln_g, ln_b):
    key = np.asarray(key, np.float32)
    mask = np.asarray(key_padding_mask).astype(bool)
    in_proj_w = np.asarray(in_proj_w, np.float32)
    in_proj_b = np.asarray(in_proj_b, np.float32)
    out_w = np.asarray(out_w, np.float32)
    out_b = np.asarray(out_b, np.float32)
    ln_g = np.asarray(ln_g, np.float32)
    ln_b = np.asarray(ln_b, np.float32)

    wq = np.ascontiguousarray(in_proj_w[:E].T)
    wk = np.ascontiguousarray(in_proj_w[E:2 * E].T)
    wv = np.ascontiguousarray(in_proj_w[2 * E:].T)
    wo = np.ascontiguousarray(out_w.T)
    bq = in_proj_b[:E].astype(ml_dtypes.bfloat16)
    bk = in_proj_b[E:2 * E].astype(ml_dtypes.bfloat16)
    bv = in_proj_b[2 * E:].astype(ml_dtypes.bfloat16)
    bo = out_b.astype(ml_dtypes.bfloat16)

    # per-chunk mask blocks (causal + pad + eye-rescue), additive -BIG
    in_maps = []
    for b in range(B):
        pad_row = np.where(mask[b], -BIG, 0.0).astype(ml_dtypes.bfloat16)
        madd = np.zeros((NT, P, 512), ml_dtypes.bfloat16)
        maddt = np.zeros((NT, P, 512), ml_dtypes.bfloat16)
        for qt in range(NT):
            W = (qt + 1) * P
            cs = ((W - 1) // 512) * 512          # last-chunk start in [q,k]
            w = W - cs
            q = qt * P + np.arange(P)[:, None]   # [P, 1]
            k = cs + np.arange(w)[None, :]       # [1, w]
            m = (k > q) | (mask[b][None, cs:W] & (k != q))
            madd[qt, :, 0:w][m] = ml_dtypes.bfloat16(-BIG)
        for kt in range(NT):
            d0 = kt * P                          # leading-chunk start in [k,q]
            w = min(512, L - d0)
            k = kt * P + np.arange(P)[:, None]   # [P, 1] (partition = k)
            q = d0 + np.arange(w)[None, :]       # [1, w]
            m = (k > q) | (mask[b][kt * P:(kt + 1) * P][:, None] & (k != q))
            maddt[kt, :, 0:w][m] = ml_dtypes.bfloat16(-BIG)
        in_maps.append({
            "xt": np.ascontiguousarray(key[b].T),
            "xres": np.ascontiguousarray(key[b] + ln_b[None, :]),
            "wq": wq, "wk": wk, "wv": wv, "wo": wo,
            "bq": bq, "bk": bk, "bv": bv, "bo": bo,
            "pad": pad_row,
            "madd": madd, "maddt": maddt,
            "g": ln_g,
        })
    return in_maps


def kernel(key, query_length, key_padding_mask, in_proj_w, in_proj_b,
           out_w, out_b, ln_g, ln_b):
    assert int(query_length) == L
    nc = _get_nc()
    in_maps = _host_prep(key, key_padding_mask, in_proj_w, in_proj_b,
                         out_w, out_b, ln_g, ln_b)
    res = run_bass_kernel_spmd(nc, in_maps, core_ids=list(range(B)))
    out = np.stack([res.results[b]["out"] for b in range(B)])
    attn = np.stack([res.results[b]["attn"] for b in range(B)])
    return out, attn



# revision 9
# speedup vs baseline: 5.6497x; 5.6497x over previous
"""Trainium2 Bass kernel for nn_EpisodeMultiheadAttentionBlock.

Data-parallel over batch: each of the 8 NeuronCores handles one batch element
(B=8).  Transfer-optimized: per core we ship only

  - xin  [1032, 1024] bf16 (~2MB): rows 0-1023 = x, 1024 = additive pad row
    (-2^96 / 0), 1025-1028 = bq/bk/bv/bo, 1029 = ln gamma, 1030 = ln beta
  - wsh  [512, 1024] bf16 (1MB): this core's slice of the packed weight
    matrix [wq^T; wk^T; wv^T; wo^T] ([4096, 1024]); an on-device AllGather
    reconstructs the full weights on every core.

Everything else (x^T via PE transposes, causal/pad/eye mask blocks via
affine_select) is derived on device.  Outputs (out, attn) are bf16.

Per core, a fused attention block: q/k/v projections (bf16 matmuls) ->
causal+pad+eye masked attention with a max-free softmax computed in BOTH
[q,k] and [k,q] orientations to avoid on-device transposes -> context ->
out projection -> LayerNorm -> residual.  attn_weights (head-mean of probs)
accumulate on the PE via diag(1/(H*l)) matmuls.  Causal structure skips
fully-masked score blocks entirely.
"""
import sys

if "/opt/trn_rl_repo" not in sys.path:
    sys.path.insert(0, "/opt/trn_rl_repo")

import numpy as np
import ml_dtypes

import concourse.bass as bass
import concourse.tile as tile
from concourse import bacc, mybir
from concourse.bass_utils import run_bass_kernel_spmd

F32 = mybir.dt.float32
F32R = mybir.dt.float32r
BF16 = mybir.dt.bfloat16
Act = mybir.ActivationFunctionType
Alu = mybir.AluOpType

B = 8
L = 1024
E = 1024
H = 16
D = E // H          # 64
P = 128
NT = L // P         # 8
NE = E // P         # 8
BIG = float(2 ** 96)
LN_EPS = 1e-5
SCALE = 1.0 / np.sqrt(D)  # 0.125

XROWS = 1032        # 1024 x + pad + 4 biases + g + ln_b
R_PAD, R_BQ, R_BK, R_BV, R_BO, R_G, R_LNB = 1024, 1025, 1026, 1027, 1028, 1029, 1030
WS = 4096 // B      # weight-shard rows per core (512)


def _chunks(start, end, step=512):
    out = []
    while start < end:
        out.append((start, min(start + step, end)))
        start += step
    return out


PSSM, PSA, PST, PSC, PEXP, PTP_B = 2, 1, 2, 1, 4, 4


def build():
    nc = bacc.Bacc("TRN2", target_bir_lowering=False, debug=False, num_devices=B)

    xin_d = nc.dram_tensor("xin", [XROWS, E], BF16, kind="ExternalInput").ap()
    wsh_d = nc.dram_tensor("wsh", [WS, E], BF16, kind="ExternalInput").ap()
    out_d = nc.dram_tensor("out", [L, E], BF16, kind="ExternalOutput").ap()
    attn_d = nc.dram_tensor("attn", [L, L], BF16, kind="ExternalOutput").ap()
    wint_d = nc.dram_tensor("wint", [WS, E], BF16, kind="Internal").ap()
    wg_d = nc.dram_tensor("wg", [4 * E, E], BF16, kind="Internal",
                          addr_space="Shared").ap()

    with tile.TileContext(nc) as tc:
        # weight shard -> internal -> AllGather to full packed weights
        nc.sync.dma_start(out=wint_d[:], in_=wsh_d[:])
        nc.gpsimd.collective_compute(
            kind="AllGather",
            op=Alu.bypass,
            replica_groups=[list(range(B))],
            ins=[wint_d[:]],
            outs=[wg_d[:]],
        )

        # ---------- long-lived constants (+ on-device mask build) ----------
        with (
            tc.tile_pool(name="consts", bufs=1) as consts,
            tc.tile_pool(name="mtmp", bufs=4) as mtmpp,
            tc.tile_pool(name="dscratch", bufs=1, space="DRAM") as dscratch,
        ):
            sdram = dscratch.tile([H, L], F32)
            ctxf_sb = consts.tile([P, NE, L], BF16)   # resident unscaled ctx^T
            ones_bf = consts.tile([1, L], BF16)
            nc.vector.memset(ones_bf[:], 1.0)
            pad_sb = consts.tile([1, L], BF16)
            nc.sync.dma_start(out=pad_sb[:], in_=xin_d[R_PAD:R_PAD + 1, :])
            bq_sb = consts.tile([1, E], BF16)
            nc.sync.dma_start(out=bq_sb[:], in_=xin_d[R_BQ:R_BQ + 1, :])
            bk_sb = consts.tile([1, E], BF16)
            nc.sync.dma_start(out=bk_sb[:], in_=xin_d[R_BK:R_BK + 1, :])
            bv_sb = consts.tile([1, E], BF16)
            nc.sync.dma_start(out=bv_sb[:], in_=xin_d[R_BV:R_BV + 1, :])
            bo_sb = consts.tile([1, E], BF16)
            nc.sync.dma_start(out=bo_sb[:], in_=xin_d[R_BO:R_BO + 1, :])
            g_bcast = consts.tile([P, E], BF16)
            nc.sync.dma_start(
                out=g_bcast[:],
                in_=bass.AP(tensor=xin_d.tensor, offset=R_G * E, ap=[[0, P], [1, E]]),
            )
            lnb_bcast = consts.tile([P, E], BF16)
            nc.sync.dma_start(
                out=lnb_bcast[:],
                in_=bass.AP(tensor=xin_d.tensor, offset=R_LNB * E, ap=[[0, P], [1, E]]),
            )
            eps_sb = consts.tile([P, 1], F32)
            nc.vector.memset(eps_sb[:], LN_EPS)
            idn = consts.tile([P, P], BF16)
            nc.vector.memset(idn[:], 1.0)
            idnm = consts.tile([P, P], BF16)
            nc.gpsimd.affine_select(
                out=idnm[:], in_=idn[:],
                pattern=[[-1, P]], base=0, channel_multiplier=1,
                compare_op=Alu.is_equal, fill=0.0,
            )
            # padk_col[p, kt] = pad[kt*P + p]
            padk_bf = consts.tile([P, NT], BF16)
            nc.sync.dma_start(
                out=padk_bf[:],
                in_=bass.AP(tensor=xin_d.tensor, offset=R_PAD * E, ap=[[1, P], [P, NT]]),
            )
            padk_col = consts.tile([P, NT], F32)
            nc.vector.tensor_copy(out=padk_col[:], in_=padk_bf[:])

            # masks for [q,k] orientation: diagonal 512-chunk per q-tile
            madd_sb = consts.tile([P, NT, 512], BF16)
            for qt in range(NT):
                W = (qt + 1) * P
                cs = ((W - 1) // 512) * 512
                w = W - cs
                m1 = mtmpp.tile([P, 512], BF16, name=f"m1q{qt}", tag="m1")
                nc.sync.dma_start(
                    out=m1[:, 0:w],
                    in_=bass.AP(tensor=xin_d.tensor, offset=R_PAD * E + cs,
                                ap=[[0, P], [1, w]]),
                )
                m2 = mtmpp.tile([P, 512], BF16, name=f"m2q{qt}", tag="m2")
                # iota = (cs - qt*P) - p + j = k - q ; k==q -> 0 (eye rescue)
                nc.gpsimd.affine_select(
                    out=m2[:, 0:w], in_=m1[:, 0:w],
                    pattern=[[1, w]], base=cs - qt * P, channel_multiplier=-1,
                    compare_op=Alu.not_equal, fill=0.0,
                )
                # keep where q - k >= 0; k > q -> -BIG (causal)
                nc.gpsimd.affine_select(
                    out=madd_sb[:, qt, 0:w], in_=m2[:, 0:w],
                    pattern=[[-1, w]], base=qt * P - cs, channel_multiplier=1,
                    compare_op=Alu.is_ge, fill=-BIG,
                )

            # masks for [k,q] orientation: leading chunk per k-tile
            maddt_sb = consts.tile([P, NT, 512], BF16)
            for kt in range(NT):
                d0 = kt * P
                w = min(512, L - d0)
                m1 = mtmpp.tile([P, 512], BF16, name=f"m1k{kt}", tag="m1")
                nc.vector.memset(m1[:, 0:w], 1.0)
                m2 = mtmpp.tile([P, 512], BF16, name=f"m2k{kt}", tag="m2")
                nc.vector.tensor_scalar_mul(
                    m2[:, 0:w], m1[:, 0:w], padk_col[:, kt:kt + 1]
                )
                # iota = p - j = k - q ; k==q -> 0 (eye rescue)
                nc.gpsimd.affine_select(
                    out=m1[:, 0:w], in_=m2[:, 0:w],
                    pattern=[[-1, w]], base=0, channel_multiplier=1,
                    compare_op=Alu.not_equal, fill=0.0,
                )
                # keep where q - k >= 0; k > q -> -BIG (causal)
                nc.gpsimd.affine_select(
                    out=maddt_sb[:, kt, 0:w], in_=m1[:, 0:w],
                    pattern=[[1, w]], base=0, channel_multiplier=-1,
                    compare_op=Alu.is_ge, fill=-BIG,
                )

            # ---------- persistent activations ----------
            with tc.tile_pool(name="acts", bufs=1) as acts:
                qt_sb = acts.tile([P, NE, L], F32R)   # [e' in tile, me, t]
                kt_sb = acts.tile([P, NE, L], F32R)
                v_sb = acts.tile([P, NT, E], BF16)    # [t in tile, mt, e']

                # ============ phase 1: x^T by PE transpose + projections ============
                with (
                    tc.tile_pool(name="p1", bufs=1) as p1,
                    tc.tile_pool(name="wstr", bufs=6) as wstr,
                ):
                    x_sb = p1.tile([P, NT, E], BF16)
                    nc.sync.dma_start(
                        out=x_sb[:],
                        in_=bass.AP(tensor=xin_d.tensor, offset=0,
                                    ap=[[E, P], [P * E, NT], [1, E]]),
                    )
                    xt_sb = p1.tile([P, NE, L], BF16)
                    with tc.tile_pool(name="pstr", bufs=2, space="PSUM") as pstr:
                        for ke in range(NE):
                            for g2 in range(2):
                                tp = pstr.tile(
                                    [P, 512], BF16, name=f"tp{ke}g{g2}", tag="tp"
                                )
                                for i in range(4):
                                    mt = 4 * g2 + i
                                    nc.tensor.transpose(
                                        tp[:, i * P:(i + 1) * P],
                                        x_sb[:, mt, ke * P:(ke + 1) * P],
                                        idnm[:],
                                    )
                                nc.scalar.copy(
                                    out=xt_sb[:, ke, g2 * 512:(g2 + 1) * 512], in_=tp[:]
                                )

                    ps1_cm = tc.tile_pool(name="ps1", bufs=4, space="PSUM")
                    ps1 = ps1_cm.__enter__()
                    wv_sb = p1.tile([P, NE, E], BF16)
                    nc.sync.dma_start(
                        out=wv_sb[:],
                        in_=bass.AP(tensor=wg_d.tensor, offset=2 * E * E,
                                    ap=[[E, P], [P * E, NE], [1, E]]),
                    )

                    for wrow, b_sb, dst in ((0, bq_sb, qt_sb), (E, bk_sb, kt_sb)):
                        for me in range(NE):
                            psc = [
                                ps1.tile([P, 512], F32, name=f"psqk{wrow}{me}c{c}",
                                         tag=f"psqk{c}")
                                for c in range(2)
                            ]
                            for ke in range(NE):
                                wt = wstr.tile([P, P], BF16, name=f"wt{wrow}{me}k{ke}",
                                               tag="wt")
                                nc.sync.dma_start(
                                    out=wt[:],
                                    in_=wg_d[wrow + ke * P:wrow + (ke + 1) * P,
                                             me * P:(me + 1) * P],
                                )
                                for c in range(2):
                                    nc.tensor.matmul(
                                        psc[c][:], wt[:],
                                        xt_sb[:, ke, c * 512:(c + 1) * 512],
                                        start=(ke == 0), stop=False,
                                    )
                            for c in range(2):
                                nc.tensor.matmul(
                                    psc[c][:],
                                    b_sb[0:1, me * P:(me + 1) * P],
                                    ones_bf[0:1, 0:512],
                                    start=False, stop=True,
                                )
                                nc.scalar.copy(
                                    out=dst[:, me, c * 512:(c + 1) * 512], in_=psc[c][:]
                                )

                    for mt in range(NT):
                        psc = [
                            ps1.tile([P, 512], F32, name=f"psv{mt}c{c}", tag=f"psqk{c}")
                            for c in range(2)
                        ]
                        for ke in range(NE):
                            for c in range(2):
                                nc.tensor.matmul(
                                    psc[c][:],
                                    xt_sb[:, ke, mt * P:(mt + 1) * P],
                                    wv_sb[:, ke, c * 512:(c + 1) * 512],
                                    start=(ke == 0), stop=False,
                                )
                        for c in range(2):
                            nc.tensor.matmul(
                                psc[c][:],
                                ones_bf[0:1, 0:P],
                                bv_sb[0:1, c * 512:(c + 1) * 512],
                                start=False, stop=True,
                            )
                            nc.scalar.copy(
                                out=v_sb[:, mt, c * 512:(c + 1) * 512], in_=psc[c][:]
                            )
                    ps1_cm.__exit__(None, None, None)

                # ======== phases 2a+2b interleaved: A-path + ST/ctx ========
                with (
                    tc.tile_pool(name="p2a", bufs=1) as p2a,
                    tc.tile_pool(name="pexp", bufs=PEXP) as pexp,
                    tc.tile_pool(name="small", bufs=8) as small,
                    tc.tile_pool(name="aout", bufs=2) as aoutp,
                    tc.tile_pool(name="ptp", bufs=PTP_B) as ptp,
                    tc.tile_pool(name="ctxu", bufs=3) as ctxup,
                    tc.tile_pool(name="psSm", bufs=PSSM, space="PSUM") as psSm,
                    tc.tile_pool(name="psA", bufs=PSA, space="PSUM") as psA,
                    tc.tile_pool(name="psT", bufs=PST, space="PSUM") as psT,
                    tc.tile_pool(name="psC", bufs=PSC, space="PSUM") as psC,
                ):
                    s_all = p2a.tile([P, H, NT], F32)
                    zeros = p2a.tile([P, L - P], BF16)
                    nc.vector.memset(zeros[:], 0.0)
                    for qt in range(NT - 1):
                        nc.sync.dma_start(
                            out=attn_d[qt * P:(qt + 1) * P, (qt + 1) * P:L],
                            in_=zeros[:, 0:L - (qt + 1) * P],
                        )

                    def block_2a(qt):
                        W = (qt + 1) * P
                        dc = qt * P
                        a_ps = psA.tile([P, L], F32, name=f"aps{qt}", tag="aps")
                        for h in range(H):
                            po = (h % 2) * 64
                            qslice = qt_sb[po:po + 64, h // 2, qt * P:(qt + 1) * P]
                            p_t = pexp.tile([P, L], BF16, name=f"pt{qt}h{h}", tag="pt")
                            ch = _chunks(0, W)
                            l_parts = small.tile(
                                [P, len(ch)], F32, name=f"lp{qt}h{h}", tag="lp"
                            )
                            for ci, (cs, ce) in enumerate(ch):
                                s_ps = psSm.tile(
                                    [P, 512], F32, name=f"sps{qt}h{h}c{ci}", tag="sps"
                                )
                                w = ce - cs
                                if ce <= dc:
                                    nc.tensor.matmul(
                                        s_ps[:, 0:w], qslice,
                                        kt_sb[po:po + 64, h // 2, cs:ce],
                                        start=True, stop=False,
                                    )
                                    nc.tensor.matmul(
                                        s_ps[:, 0:w],
                                        ones_bf[0:1, 0:P],
                                        pad_sb[0:1, cs:ce],
                                        start=False, stop=True,
                                    )
                                else:
                                    nc.tensor.matmul(
                                        s_ps[:, 0:w], qslice,
                                        kt_sb[po:po + 64, h // 2, cs:ce],
                                        start=True, stop=True,
                                    )
                                    nc.vector.tensor_add(
                                        out=s_ps[:, 0:w], in0=s_ps[:, 0:w],
                                        in1=madd_sb[:, qt, 0:w],
                                    )
                                nc.scalar.activation(
                                    out=p_t[:, cs:ce], in_=s_ps[:, 0:w],
                                    func=Act.Exp, scale=SCALE,
                                    accum_out=l_parts[:, ci:ci + 1],
                                )
                            l16 = small.tile([P, 1], F32, name=f"l16{qt}h{h}", tag="l16")
                            if len(ch) > 1:
                                l_s = small.tile([P, 1], F32, name=f"ls{qt}h{h}", tag="ls")
                                nc.vector.tensor_reduce(
                                    out=l_s[:], in_=l_parts[:],
                                    axis=mybir.AxisListType.X, op=Alu.add,
                                )
                                nc.vector.tensor_scalar_mul(l16[:], l_s[:], 16.0)
                            else:
                                nc.vector.tensor_scalar_mul(l16[:], l_parts[:, 0:1], 16.0)
                            s_col = small.tile([P, 1], F32, name=f"sc{qt}h{h}", tag="sc")
                            nc.vector.reciprocal(out=s_col[:], in_=l16[:])
                            nc.vector.tensor_copy(
                                out=s_all[:, h, qt:qt + 1], in_=s_col[:]
                            )
                            dg = small.tile([P, P], BF16, name=f"dg{qt}h{h}", tag="dg")
                            nc.vector.tensor_scalar_mul(dg[:], idnm[:], s_col[:])
                            for (cs, ce) in _chunks(0, W):
                                nc.tensor.matmul(
                                    a_ps[:, cs:ce], dg[:], p_t[:, cs:ce],
                                    start=(h == 0), stop=(h == H - 1),
                                )
                        a_out = aoutp.tile([P, L], BF16, name=f"aout{qt}", tag="aout")
                        nc.scalar.copy(out=a_out[:, 0:W], in_=a_ps[:, 0:W])
                        nc.sync.dma_start(
                            out=attn_d[qt * P:(qt + 1) * P, 0:W], in_=a_out[:, 0:W]
                        )

                    def block_2b(h):
                        ctx_ps = psC.tile([64, L], F32, name=f"ctxps{h}", tag="ctxps")
                        po = (h % 2) * 64
                        for kt in range(NT):
                            c0 = 512 * (kt // 4)
                            d0 = kt * P
                            kslice = kt_sb[po:po + 64, h // 2, kt * P:(kt + 1) * P]
                            pt_t = ptp.tile(
                                [P, L], BF16, name=f"ptt{h}k{kt}", tag="ptt"
                            )
                            if c0 < d0:
                                nc.vector.memset(pt_t[:, c0:d0], 0.0)
                            for ci, (cs, ce) in enumerate(_chunks(d0, L)):
                                st_ps = psT.tile(
                                    [P, 512], F32, name=f"stp{h}k{kt}c{ci}", tag="stp"
                                )
                                w = ce - cs
                                if ci == 0:
                                    nc.tensor.matmul(
                                        st_ps[:, 0:w], kslice,
                                        qt_sb[po:po + 64, h // 2, cs:ce],
                                        start=True, stop=True,
                                    )
                                    nc.vector.tensor_add(
                                        out=st_ps[:, 0:w], in0=st_ps[:, 0:w],
                                        in1=maddt_sb[:, kt, 0:w],
                                    )
                                else:
                                    nc.tensor.matmul(
                                        st_ps[:, 0:w], kslice,
                                        qt_sb[po:po + 64, h // 2, cs:ce],
                                        start=True, stop=False,
                                    )
                                    nc.tensor.matmul(
                                        st_ps[:, 0:w],
                                        pad_sb[0:1, kt * P:(kt + 1) * P],
                                        ones_bf[0:1, cs:ce],
                                        start=False, stop=True,
                                    )
                                nc.scalar.activation(
                                    out=pt_t[:, cs:ce], in_=st_ps[:, 0:w],
                                    func=Act.Exp, scale=SCALE,
                                )
                            for (cs, ce) in _chunks(0, L):
                                if ce <= c0:
                                    continue
                                n_kt = min(NT, (ce + P - 1) // P)
                                nc.tensor.matmul(
                                    ctx_ps[:, cs:ce],
                                    v_sb[:, kt, h * D:(h + 1) * D],
                                    pt_t[:, cs:ce],
                                    start=(kt == 0),
                                    stop=(kt == n_kt - 1),
                                )
                        ctxu = ctxup.tile([64, L], BF16, name=f"ctxu{h}", tag="ctxu")
                        po2 = (h % 2) * 64
                        for (cs, ce) in _chunks(0, L):
                            nc.scalar.copy(out=ctxu[:, cs:ce], in_=ctx_ps[:, cs:ce])
                            nc.sync.dma_start(
                                out=ctxf_sb[po2:po2 + 64, h // 2, cs:ce],
                                in_=ctxu[:, cs:ce],
                            )

                    for i in range(NT):
                        block_2a(i)
                        block_2b(2 * i)
                        block_2b(2 * i + 1)
                    nc.sync.dma_start(
                        out=bass.AP(
                            tensor=sdram.tensor, offset=sdram.offset,
                            ap=[[1, P], [L, H], [P, NT]],
                        ),
                        in_=s_all[:],
                    )

            # ================= phase 3: out-proj + LN + residual =================
            with (
                tc.tile_pool(name="p3", bufs=1) as p3,
                tc.tile_pool(name="xrp", bufs=2) as xrp,
                tc.tile_pool(name="znp", bufs=2) as znp,
                tc.tile_pool(name="lns", bufs=8) as lns,
                tc.tile_pool(name="ps3", bufs=4, space="PSUM") as ps3,
            ):
                ctxt_sb = p3.tile([P, NE, L], BF16)
                for ke in range(NE):
                    sbc2 = xrp.tile([P, L], F32, name=f"sbc2k{ke}", tag="sbc2")
                    nc.sync.dma_start(
                        out=sbc2[:],
                        in_=bass.AP(
                            tensor=sdram.tensor, offset=sdram.offset + 2 * ke * L,
                            ap=[[L, 2], [0, 64], [1, L]],
                        ),
                    )
                    nc.vector.scalar_tensor_tensor(
                        out=ctxt_sb[:, ke, :], in0=ctxf_sb[:, ke, :],
                        scalar=16.0, in1=sbc2[:],
                        op0=Alu.mult, op1=Alu.mult,
                    )
                wo_sb = p3.tile([P, NE, E], BF16)
                nc.sync.dma_start(
                    out=wo_sb[:],
                    in_=bass.AP(tensor=wg_d.tensor, offset=3 * E * E,
                                ap=[[E, P], [P * E, NE], [1, E]]),
                )
                for qt in range(NT):
                    psc = [
                        ps3.tile([P, 512], F32, name=f"pso{qt}c{c}", tag=f"pso{c}")
                        for c in range(2)
                    ]
                    for ke in range(NE):
                        for c in range(2):
                            nc.tensor.matmul(
                                psc[c][:],
                                ctxt_sb[:, ke, qt * P:(qt + 1) * P],
                                wo_sb[:, ke, c * 512:(c + 1) * 512],
                                start=(ke == 0), stop=False,
                            )
                    for c in range(2):
                        nc.tensor.matmul(
                            psc[c][:],
                            ones_bf[0:1, 0:P],
                            bo_sb[0:1, c * 512:(c + 1) * 512],
                            start=False, stop=True,
                        )
                    stats = lns.tile([P, 2, 6], F32, name=f"st{qt}", tag="st")
                    for c in range(2):
                        nc.vector.bn_stats(out=stats[:, c, :], in_=psc[c][:])
                    mv = lns.tile([P, 2], F32, name=f"mv{qt}", tag="mv")
                    nc.vector.bn_aggr(out=mv[:], in_=stats[:])
                    sd = lns.tile([P, 1], F32, name=f"sd{qt}", tag="sd")
                    nc.scalar.activation(
                        out=sd[:], in_=mv[:, 1:2], func=Act.Sqrt, bias=eps_sb[:],
                    )
                    rstd = lns.tile([P, 1], F32, name=f"rs{qt}", tag="rs")
                    nc.vector.reciprocal(out=rstd[:], in_=sd[:])
                    nmu = lns.tile([P, 1], F32, name=f"nm{qt}", tag="nm")
                    nc.vector.scalar_tensor_tensor(
                        out=nmu[:], in0=mv[:, 0:1], scalar=-1.0, in1=rstd[:],
                        op0=Alu.mult, op1=Alu.mult,
                    )
                    zn = znp.tile([P, E], F32, name=f"zn{qt}", tag="zn")
                    for c in range(2):
                        nc.scalar.activation(
                            out=zn[:, c * 512:(c + 1) * 512], in_=psc[c][:],
                            func=Act.Identity, bias=nmu[:], scale=rstd[:],
                        )
                    xr = xrp.tile([P, E], BF16, name=f"xr{qt}", tag="xr")
                    nc.sync.dma_start(out=xr[:], in_=xin_d[qt * P:(qt + 1) * P, :])
                    nc.vector.tensor_mul(zn[:], zn[:], g_bcast[:])
                    nc.vector.tensor_add(zn[:], zn[:], lnb_bcast[:])
                    zb = znp.tile([P, E], BF16, name=f"zb{qt}", tag="zb")
                    nc.vector.tensor_add(zb[:], zn[:], xr[:])
                    nc.sync.dma_start(out=out_d[qt * P:(qt + 1) * P, :], in_=zb[:])

    nc.compile()
    return nc


_NC = None


def _get_nc():
    global _NC
    if _NC is None:
        _NC = build()
    return _NC


def _host_prep(key, key_padding_mask, in_proj_w, in_proj_b, out_w, out_b, ln_g, ln_b):
    key = np.asarray(key, np.float32)
    mask = np.asarray(key_padding_mask).astype(bool)
    in_proj_w = np.asarray(in_proj_w, np.float32)
    in_proj_b = np.asarray(in_proj_b, np.float32)
    out_w = np.asarray(out_w, np.float32)
    out_b = np.asarray(out_b, np.float32)
    ln_g = np.asarray(ln_g, np.float32)
    ln_b = np.asarray(ln_b, np.float32)

    wpack = np.concatenate(
        [in_proj_w[:E].T, in_proj_w[E:2 * E].T, in_proj_w[2 * E:].T, out_w.T],
        axis=0,
    ).astype(ml_dtypes.bfloat16)                      # [4096, 1024]

    in_maps = []
    for b in range(B):
        xin = np.zeros((XROWS, E), ml_dtypes.bfloat16)
        xin[0:L] = key[b]
        xin[R_PAD] = np.where(mask[b], -BIG, 0.0)
        xin[R_BQ] = in_proj_b[:E]
        xin[R_BK] = in_proj_b[E:2 * E]
        xin[R_BV] = in_proj_b[2 * E:]
        xin[R_BO] = out_b
        xin[R_G] = ln_g
        xin[R_LNB] = ln_b
        in_maps.append({
            "xin": xin,
            "wsh": np.ascontiguousarray(wpack[b * WS:(b + 1) * WS]),
        })
    return in_maps


def kernel(key, query_length, key_padding_mask, in_proj_w, in_proj_b,
           out_w, out_b, ln_g, ln_b):
    assert int(query_length) == L
    nc = _get_nc()
    in_maps = _host_prep(key, key_padding_mask, in_proj_w, in_proj_b,
                         out_w, out_b, ln_g, ln_b)
    res = run_bass_kernel_spmd(nc, in_maps, core_ids=list(range(B)))
    out = np.stack([res.results[b]["out"].astype(np.float32) for b in range(B)])
    attn = np.stack([res.results[b]["attn"].astype(np.float32) for b in range(B)])
    return out, attn
